# revision 3
# baseline (speedup 1.0000x reference)
"""GCNII (8-layer GCN2Conv stack) on 8 Trainium2 NeuronCores.

Strategy (1D graph parallel over destination nodes):
  - Core c owns destination rows [c*NS, (c+1)*NS) of every layer's output.
  - Edges are partitioned by dst, sorted by dst, grouped into per-128-dst-node
    "blocks", padded to 128-edge chunks with zero-weight edges.
  - Per chunk: indirect-DMA row gather of h[src] (128 rows, 512B each) into
    SBUF, a one-hot selector S[e, j] = (j == dstloc[e]) * w[e] built on the
    vector engine from a resident iota tile, and a tensor-engine matmul
    aggT += V.T @ S accumulated in PSUM (feature-major: [feat, dst]).
  - Block epilogue: mT = aggT + 0.1*x0T (vector), node-major
    h' = relu(mT.T @ W_hat_l) via one matmul (W_hat_l = beta_l*W_l +
    (1-beta_l)*I folded on host; 0.9 folded into edge weights) + scalar-engine
    relu, DMA to the layer's shard buffer.
  - AllGather of the 6250-row shard between layers gives every core the full
    h for the next layer's gathers. Final layer writes the external output
    directly (host concatenates shards).
"""

import math
import numpy as np

N = 50000
E = 800000
D = 128
L = 8
ALPHA = 0.1
THETA = 0.5
NCORES = 8
P = 128

_NEFF_CACHE = {}
_LAST_IN_MAPS = None


def _preprocess(edge_index, edge_weight, n, ns, nb, ncores):
    """Sort/partition edges; build per-core [128, NCH] chunk arrays."""
    src = np.asarray(edge_index[0], dtype=np.int64)
    dst = np.asarray(edge_index[1], dtype=np.int64)
    w = np.asarray(edge_weight, dtype=np.float32) * (1.0 - ALPHA)

    order = np.argsort(dst, kind="stable")
    src_s, dst_s, w_s = src[order], dst[order], w[order]

    core = dst_s // ns
    local = dst_s - core * ns
    block = local // P
    dstloc = local - block * P

    # per (core, block) edge counts -> shared chunks-per-block schedule
    cb = (core * nb + block).astype(np.int64)
    counts = np.bincount(cb, minlength=ncores * nb).reshape(ncores, nb)
    cpb = np.maximum(1, -(-counts.max(axis=0) // P))  # ceil, >=1
    nch = int(cpb.sum())
    chunk_off = np.concatenate([[0], np.cumsum(cpb)[:-1]])  # chunk idx of block b

    # block boundaries within the sorted edge list, per core
    src_arrs, dl_arrs, w_arrs = [], [], []
    core_starts = np.searchsorted(core, np.arange(ncores + 1))
    for c in range(ncores):
        s0, s1 = core_starts[c], core_starts[c + 1]
        blk = block[s0:s1]
        bstart = np.searchsorted(blk, np.arange(nb + 1)) + s0
        sa = np.zeros(nch * P, dtype=np.int32)
        da = np.zeros(nch * P, dtype=np.float32)
        wa = np.zeros(nch * P, dtype=np.float32)
        for b in range(nb):
            e0, e1 = bstart[b], bstart[b + 1]
            cnt = e1 - e0
            pos = chunk_off[b] * P
            sa[pos:pos + cnt] = src_s[e0:e1]
            da[pos:pos + cnt] = dstloc[e0:e1]
            wa[pos:pos + cnt] = w_s[e0:e1]
        src_arrs.append(np.ascontiguousarray(sa.reshape(nch, P).T))
        dl_arrs.append(np.ascontiguousarray(da.reshape(nch, P).T))
        w_arrs.append(np.ascontiguousarray(wa.reshape(nch, P).T))

    chunk_block = np.repeat(np.arange(nb), cpb)  # block index of each chunk
    return src_arrs, dl_arrs, w_arrs, nch, chunk_block


def _build(nc, *, n, ns, nb, nch, chunk_block, n_layers):
    import concourse.bass as bass
    import concourse.mybir as mybir
    import concourse.tile as tile
    from concourse.masks import make_identity

    f32 = mybir.dt.float32
    i32 = mybir.dt.int32

    x_t = nc.dram_tensor("x_shard", [ns, D], f32, kind="ExternalInput")
    wlin_t = nc.dram_tensor("w_lin", [D, D], f32, kind="ExternalInput")
    blin_t = nc.dram_tensor("b_lin", [D], f32, kind="ExternalInput")
    what_t = nc.dram_tensor("w_hat", [n_layers, D, D], f32, kind="ExternalInput")
    srcidx_t = nc.dram_tensor("src_idx", [P, nch], i32, kind="ExternalInput")
    dstloc_t = nc.dram_tensor("dst_loc", [P, nch], f32, kind="ExternalInput")
    wgt_t = nc.dram_tensor("wgt", [P, nch], f32, kind="ExternalInput")
    out_t = nc.dram_tensor("h_out", [ns, D], f32, kind="ExternalOutput")

    # internal DRAM: ping-pong full-h buffers (AllGather outputs must be
    # Shared) and shard buffers (AllGather inputs must be Local).
    hbuf = [nc.dram_tensor(f"h_full{i}", [n, D], f32, addr_space="Shared")
            for i in range(2)]
    shbuf = [nc.dram_tensor(f"h_shard{i}", [ns, D], f32) for i in range(2)]
    rg = [list(range(NCORES))]

    n_full = ns // P            # number of full 128-row blocks
    last = ns - n_full * P      # rows in last partial block (0 if none)
    blk_rows = [P] * n_full + ([last] if last else [])
    assert len(blk_rows) == nb

    with tile.TileContext(nc) as tc:
        with (
            tc.tile_pool(name="res", bufs=1) as res,
            tc.tile_pool(name="xp", bufs=3) as xp,
            tc.tile_pool(name="vp", bufs=16) as vp,
            tc.tile_pool(name="sp", bufs=8) as sp,
            tc.tile_pool(name="mp", bufs=3) as mp,
            tc.tile_pool(name="hp", bufs=4) as hp,
            tc.tile_pool(name="ps", bufs=2, space="PSUM") as ps,
        ):
            ident = res.tile([P, P], f32, tag="ident")
            make_identity(nc, ident[:])
            iota_i = res.tile([P, P], i32, tag="iotai")
            nc.gpsimd.iota(iota_i[:], pattern=[[1, P]], base=0,
                           channel_multiplier=0)
            iota_f = res.tile([P, P], f32, tag="iotaf")
            nc.vector.tensor_copy(out=iota_f[:], in_=iota_i[:])

            wlin_s = res.tile([P, D], f32, tag="wlin")
            nc.sync.dma_start(out=wlin_s[:], in_=wlin_t[:])
            blin_s = res.tile([P, 1], f32, tag="blin")
            nc.sync.dma_start(out=blin_s[:], in_=blin_t[:, None])
            what_s = res.tile([P, n_layers * D], f32, tag="what")
            for l in range(n_layers):
                nc.sync.dma_start(out=what_s[:, l * D:(l + 1) * D],
                                  in_=what_t[l, :, :])
            srcix = res.tile([P, nch], i32, tag="srcix")
            nc.sync.dma_start(out=srcix[:], in_=srcidx_t[:])
            dstloc = res.tile([P, nch], f32, tag="dstloc")
            nc.sync.dma_start(out=dstloc[:], in_=dstloc_t[:])
            wgt = res.tile([P, nch], f32, tag="wgt")
            nc.sync.dma_start(out=wgt[:], in_=wgt_t[:])

            x0sT = res.tile([P, ns], f32, tag="x0sT")  # 0.1 * relu(xW+b).T

            # ---- prologue: x0 = relu(x @ W_lin + b) ----
            for b in range(nb):
                rows = blk_rows[b]
                r0 = b * P
                xb = xp.tile([P, D], f32, tag="xb")
                nc.sync.dma_start(out=xb[:rows, :], in_=x_t[r0:r0 + rows, :])
                xbT_ps = ps.tile([P, P], f32, tag="tr", space="PSUM")
                nc.tensor.transpose(out=xbT_ps[:, :rows], in_=xb[:rows, :],
                                    identity=ident[:rows, :rows])
                xbT = xp.tile([P, P], f32, tag="xbT")
                nc.vector.tensor_copy(out=xbT[:, :rows], in_=xbT_ps[:, :rows])
                ps2 = ps.tile([P, P], f32, tag="dense", space="PSUM")
                nc.tensor.matmul(out=ps2[:, :rows], lhsT=wlin_s[:],
                                 rhs=xbT[:, :rows], start=True, stop=True)
                x0Tb = xp.tile([P, P], f32, tag="x0Tb")
                nc.scalar.activation(out=x0Tb[:, :rows], in_=ps2[:, :rows],
                                     func=mybir.ActivationFunctionType.Relu,
                                     bias=blin_s[:, :1], scale=1.0)
                nc.vector.tensor_scalar(
                    out=x0sT[:, r0:r0 + rows], in0=x0Tb[:, :rows],
                    scalar1=ALPHA, scalar2=None, op0=mybir.AluOpType.mult)
                x0_ps = ps.tile([P, P], f32, tag="tr", space="PSUM")
                nc.tensor.transpose(out=x0_ps[:rows, :], in_=x0Tb[:, :rows],
                                    identity=ident[:])
                x0b = hp.tile([P, D], f32, tag="hb")
                nc.vector.tensor_copy(out=x0b[:rows, :], in_=x0_ps[:rows, :])
                nc.sync.dma_start(out=shbuf[0][r0:r0 + rows, :],
                                  in_=x0b[:rows, :])

            nc.gpsimd.collective_compute(
                "AllGather", mybir.AluOpType.bypass, replica_groups=rg,
                ins=[shbuf[0][:]], outs=[hbuf[0][:]])

            # ---- layers ----
            for l in range(n_layers):
                h_cur = hbuf[l % 2]
                is_last = l == n_layers - 1
                k = 0
                for b in range(nb):
                    rows = blk_rows[b]
                    r0 = b * P
                    aggT = ps.tile([P, P], f32, tag="agg", space="PSUM")
                    k0 = k
                    while k < nch and chunk_block[k] == b:
                        v = vp.tile([P, D], f32, tag="v")
                        nc.gpsimd.indirect_dma_start(
                            out=v[:], out_offset=None, in_=h_cur[:],
                            in_offset=bass.IndirectOffsetOnAxis(
                                ap=srcix[:, k:k + 1], axis=0))
                        s = sp.tile([P, P], f32, tag="s")
                        nc.vector.tensor_scalar(
                            out=s[:], in0=iota_f[:],
                            scalar1=dstloc[:, k:k + 1],
                            scalar2=wgt[:, k:k + 1],
                            op0=mybir.AluOpType.is_equal,
                            op1=mybir.AluOpType.mult)
                        is_first = k == k0
                        is_stop = k + 1 >= nch or chunk_block[k + 1] != b
                        nc.tensor.matmul(out=aggT[:], lhsT=v[:], rhs=s[:],
                                         start=is_first, stop=is_stop)
                        k += 1
                    mT = mp.tile([P, P], f32, tag="mT")
                    nc.vector.tensor_tensor(
                        out=mT[:, :rows], in0=aggT[:, :rows],
                        in1=x0sT[:, r0:r0 + rows], op=mybir.AluOpType.add)
                    ps2 = ps.tile([P, P], f32, tag="dense", space="PSUM")
                    nc.tensor.matmul(out=ps2[:rows, :], lhsT=mT[:, :rows],
                                     rhs=what_s[:, l * D:(l + 1) * D],
                                     start=True, stop=True)
                    hb = hp.tile([P, D], f32, tag="hb")
                    nc.scalar.activation(out=hb[:rows, :], in_=ps2[:rows, :],
                                         func=mybir.ActivationFunctionType.Relu)
                    dst_dram = out_t if is_last else shbuf[(l + 1) % 2]
                    nc.sync.dma_start(out=dst_dram[r0:r0 + rows, :],
                                      in_=hb[:rows, :])
                if not is_last:
                    nc.gpsimd.collective_compute(
                        "AllGather", mybir.AluOpType.bypass, replica_groups=rg,
                        ins=[shbuf[(l + 1) % 2][:]], outs=[hbuf[(l + 1) % 2][:]])
    return nc


def _run(inputs, *, n, e, n_layers, ncores=NCORES):
    import concourse.bacc as bacc
    from concourse.bass_utils import run_bass_kernel_spmd

    x = np.asarray(inputs["x"], dtype=np.float32)
    edge_weight = np.asarray(inputs["edge_weight"], dtype=np.float32)
    w_lin = np.asarray(inputs["W_lin"], dtype=np.float32)
    b_lin = np.asarray(inputs["b_lin"], dtype=np.float32)
    w_convs = np.asarray(inputs["W_convs"], dtype=np.float32)
    edge_index = np.asarray(inputs["edge_index"])

    ns = n // ncores
    nb = -(-ns // P)

    betas = np.log(THETA / np.arange(1, n_layers + 1) + 1.0).astype(np.float32)
    eye = np.eye(D, dtype=np.float32)
    w_hat = np.stack([betas[l] * w_convs[l] + (1.0 - betas[l]) * eye
                      for l in range(n_layers)]).astype(np.float32)

    src_arrs, dl_arrs, w_arrs, nch, chunk_block = _preprocess(
        edge_index, edge_weight, n, ns, nb, ncores)

    key = (n, e, n_layers, nch, tuple(chunk_block[:64]))
    if key not in _NEFF_CACHE:
        nc = bacc.Bacc("TRN2", target_bir_lowering=False, debug=False,
                       num_devices=ncores)
        _build(nc, n=n, ns=ns, nb=nb, nch=nch, chunk_block=chunk_block,
               n_layers=n_layers)
        nc.compile()
        _NEFF_CACHE[key] = nc
    nc = _NEFF_CACHE[key]

    in_maps = []
    for c in range(ncores):
        in_maps.append({
            "x_shard": np.ascontiguousarray(x[c * ns:(c + 1) * ns]),
            "w_lin": w_lin, "b_lin": b_lin, "w_hat": w_hat,
            "src_idx": src_arrs[c], "dst_loc": dl_arrs[c], "wgt": w_arrs[c],
        })
    global _LAST_IN_MAPS
    _LAST_IN_MAPS = in_maps
    res = run_bass_kernel_spmd(nc, in_maps, list(range(ncores)))
    out = np.concatenate([res.results[c]["h_out"] for c in range(ncores)],
                         axis=0)
    return out


def kernel(x, edge_weight, W_lin, b_lin, W_convs, edge_index):
    return _run(
        dict(x=x, edge_weight=edge_weight, W_lin=W_lin, b_lin=b_lin,
             W_convs=W_convs, edge_index=edge_index),
        n=N, e=E, n_layers=L)
